# revision 19
# baseline (speedup 1.0000x reference)
"""Tensor-parallel attention kernel for trn2 (8 cores).

TP over heads (2/core) for QKV + attention; per-head AllToAll reshards
attention output to seq-parallel; output projection seq-sharded (each
core owns 256 output rows); host concatenates.

v4: all-fp16 data path. Mask applied as exp(mask) (host-precomputed)
multiplicatively after the ACT exp. Softmax sums split between DVE and
GpSimd; fused all-ones partition-reduce+broadcast matmul. Emission
order interleaves head-0 attention into the tail of the QKV phase so
the PE never drains; x is DMAed in 8 grouped transfers per half;
dedicated DMA queues (x+masks: sync/scalar HWDGE, wo: scalar,
a2a/at/y: gpsimd SWDGE); wo prefetched during attention.
"""
import math
import numpy as np

import concourse.bass as bass
import concourse.mybir as mybir
import concourse.tile as tile
from concourse import bacc
from concourse.masks import make_identity

f32 = mybir.dt.float32
f16 = mybir.dt.float16

P = 128
S = 2048
D = 2048
HD = 128
NH = 2          # heads per core
W = 8           # cores
QS = S // W     # 256 output rows per core
DT = D // P     # 16 contraction tiles
KT = S // P     # 16 kv tiles
QB = 1024       # q block width
NB = S // QB    # q blocks per head
XG = 8          # x DMA groups per half
XW = DT // XG   # d-tiles per group
HALF = S // 2

Exp = mybir.ActivationFunctionType.Exp
ADD = mybir.AluOpType.add
MULT = mybir.AluOpType.mult


def build():
    nc = bacc.Bacc("TRN2", target_bir_lowering=False, debug=False, num_devices=W)

    xt = nc.dram_tensor("xt", [2, XG, P, XW, HALF], f16,
                        kind="ExternalInput").ap()
    wq_t = nc.dram_tensor("wq_t", [NH, P, DT, HD], f16, kind="ExternalInput").ap()
    wk_t = nc.dram_tensor("wk_t", [NH, P, DT, HD], f16, kind="ExternalInput").ap()
    wv_t = nc.dram_tensor("wv_t", [NH, P, DT, HD], f16, kind="ExternalInput").ap()
    # emask = exp(mask), transposed to [NH, S_kv, S_q]
    emask_t = nc.dram_tensor("emask_t", [NH, S, S], f16, kind="ExternalInput").ap()
    wo_t = nc.dram_tensor("wo_t", [D, D], f16, kind="ExternalInput").ap()
    y = nc.dram_tensor("y", [QS, D], f32, kind="ExternalOutput").ap()

    wqkv = {"q": wq_t, "k": wk_t, "v": wv_t}

    with tile.TileContext(nc) as tc:
        persist = tc.alloc_tile_pool(name="persist", bufs=1)
        consts = tc.alloc_tile_pool(name="consts", bufs=1)
        probsp = tc.alloc_tile_pool(name="probsp", bufs=5)
        smallp = tc.alloc_tile_pool(name="smallp", bufs=2)
        dram = tc.alloc_tile_pool(name="dram", bufs=1, space="DRAM")
        maskp = tc.alloc_tile_pool(name="maskp", bufs=6)
        vpool = tc.alloc_tile_pool(name="vpool", bufs=1)
        dpool = tc.alloc_tile_pool(name="dpool", bufs=1)
        psS = tc.alloc_tile_pool(name="psS", bufs=2, space="PSUM")
        psPV = tc.alloc_tile_pool(name="psPV", bufs=1, space="PSUM")
        psA = tc.alloc_tile_pool(name="psA", bufs=2, space="PSUM")
        wpool = tc.alloc_tile_pool(name="wpool", bufs=6)
        xpool = tc.alloc_tile_pool(name="xpool", bufs=1)

        ident_f = consts.tile([P, P], f32, name="ident_f")
        make_identity(nc, ident_f[:])
        ident_h = consts.tile([P, P], f16, name="ident_h")
        nc.vector.tensor_copy(out=ident_h[:], in_=ident_f[:])
        allones_f = consts.tile([P, P], f32, name="allones_f")
        nc.gpsimd.memset(allones_f[:], 1.0)
        allones = consts.tile([P, P], f16, name="allones")
        nc.vector.tensor_copy(out=allones[:], in_=allones_f[:])

        qT = [persist.tile([P, S], f16, name=f"qT{b}") for b in range(NH)]
        kT = [persist.tile([P, S], f16, name=f"kT{b}") for b in range(NH)]
        vT = [persist.tile([P, S], f16, name=f"vT{b}") for b in range(NH)]
        v_sb = [vpool.tile([P, KT, P], f16, name=f"v_sb{b}") for b in range(NH)]

        a2a_in = [dram.tile([W, P, QS], f16, name=f"a2a_in{b}") for b in range(NH)]
        a2a_out = [dram.tile([W, P, QS], f16, name=f"a2a_out{b}") for b in range(NH)]

        dests = {"q": qT, "k": kT, "v": vT}
        dma_engines = [nc.sync, nc.scalar]

        # ---------------- Phase A helpers ----------------
        wtiles = {}
        for kind in ("k", "v", "q"):
            for b in range(NH):
                wt = wpool.tile([P, DT, HD], f16, name=f"w_{kind}{b}", tag="wt")
                nc.gpsimd.dma_start(wt[:], wqkv[kind][b])
                wtiles[(kind, b)] = wt

        def load_half(half):
            groups = []
            for g in range(XG):
                xg = xpool.tile([P, XW, HALF], f16, name=f"xg{g}", tag=f"xg{g}")
                dma_engines[g % 2].dma_start(xg[:], xt[half, g])
                groups.append(xg)
            return lambda t: groups[t // XW][:, t % XW, :]

        def proj(kind, b, xof, col0):
            wt = wtiles[(kind, b)]
            dst = dests[kind][b]
            for c in range(2):
                ps = psA.tile([P, 512], f32, name="psa", tag="psa")
                for t in range(DT):
                    nc.tensor.matmul(ps[:], wt[:, t, :],
                                     xof(t)[:, c * 512:(c + 1) * 512],
                                     start=(t == 0), stop=(t == DT - 1))
                nc.scalar.copy(dst[:, col0 + c * 512:col0 + (c + 1) * 512],
                               ps[:])

        def transp(b):
            with nc.named_scope("T"):
                for t in range(KT):
                    pst = psS.tile([P, P], f16, name="pst", tag="sc")
                    nc.tensor.transpose(pst[:], vT[b][:, t * P:(t + 1) * P],
                                        ident_h[:])
                    nc.vector.tensor_copy(out=v_sb[b][:, t, :], in_=pst[:])

        # ---------------- Phase B helper ----------------
        def attn_blk(b, blk):
          with nc.named_scope(f"B{b}_{blk}"):
            base = blk * QB
            mts = []
            for t in range(KT):
                mtile = maskp.tile([P, QB], f16, name=f"mt{t}", tag="mt")
                nc.sync.dma_start(
                    mtile[:], emask_t[b, t * P:(t + 1) * P, base:base + QB])
                mts.append(mtile)
            pv = psPV.tile([P, QB], f32, name="pv", tag="pv")
            sacc_d = smallp.tile([P, QB], f16, name="sacc_d", tag="sacc_d")
            sacc_g = smallp.tile([P, QB], f16, name="sacc_g", tag="sacc_g")
            PIPE = 2
            pq = {}
            for tt in range(KT + PIPE):
                if tt < KT:
                    t = tt
                    ktile = kT[b][:, t * P:(t + 1) * P]
                    ps_s = psS.tile([P, QB], f32, name="ps_s", tag="sc")
                    nc.tensor.matmul(ps_s[:, 0:512], ktile,
                                     qT[b][:, base:base + 512],
                                     start=True, stop=True)
                    nc.tensor.matmul(ps_s[:, 512:QB], ktile,
                                     qT[b][:, base + 512:base + QB],
                                     start=True, stop=True)
                    pr = probsp.tile([P, QB], f16, name="pr", tag="probs")
                    nc.scalar.activation(pr[:], ps_s[:], Exp)
                    nc.vector.tensor_tensor(out=pr[:], in0=pr[:],
                                            in1=mts[t][:], op=MULT)
                    if t % 2 == 0:
                        if t == 0:
                            nc.gpsimd.tensor_copy(out=sacc_g[:], in_=pr[:])
                        else:
                            nc.gpsimd.tensor_tensor(out=sacc_g[:],
                                                    in0=sacc_g[:],
                                                    in1=pr[:], op=ADD)
                    else:
                        if t == 1:
                            nc.vector.tensor_copy(out=sacc_d[:], in_=pr[:])
                        else:
                            nc.vector.tensor_tensor(out=sacc_d[:],
                                                    in0=sacc_d[:],
                                                    in1=pr[:], op=ADD)
                    pq[t] = pr
                if tt >= PIPE:
                    t = tt - PIPE
                    pr = pq.pop(t)
                    vtile = v_sb[b][:, t, :]
                    nc.tensor.matmul(pv[:, 0:512], vtile, pr[:, 0:512],
                                     start=(t == 0), stop=(t == KT - 1))
                    nc.tensor.matmul(pv[:, 512:QB], vtile, pr[:, 512:QB],
                                     start=(t == 0), stop=(t == KT - 1))
            # fused partition-reduce + broadcast of softmax sums
            ps_bc = psS.tile([P, QB], f32, name="ps_bc", tag="sc")
            for h0 in range(2):
                sl = slice(h0 * 512, (h0 + 1) * 512)
                nc.tensor.matmul(ps_bc[:, sl], allones[:], sacc_d[:, sl],
                                 start=True, stop=False)
                nc.tensor.matmul(ps_bc[:, sl], allones[:], sacc_g[:, sl],
                                 start=False, stop=True)
            bc_sb = smallp.tile([P, QB], f32, name="bc_sb", tag="bc_sb")
            nc.vector.reciprocal_approx_fast(out=bc_sb[:], in_=ps_bc[:])
            attn_sb = smallp.tile([P, QB], f16, name="attn_sb", tag="attn_sb")
            nc.vector.tensor_tensor(out=attn_sb[:], in0=pv[:],
                                    in1=bc_sb[:], op=MULT)
            for jj in range(QB // QS):
                j = blk * (QB // QS) + jj
                nc.gpsimd.dma_start(a2a_in[b][j, :, :],
                                    attn_sb[:, jj * QS:(jj + 1) * QS])

        ats = {}

        def a2a(b):
            nc.gpsimd.collective_compute(
                "AllToAll", mybir.AluOpType.bypass,
                replica_groups=[list(range(W))],
                ins=[a2a_in[b].opt()],
                outs=[a2a_out[b].opt()],
            )
            for i in range(W):
                at = dpool.tile([P, QS], f16, name=f"at{b}_{i}",
                                tag=f"at{b}_{i}")
                nc.gpsimd.dma_start(at[:], a2a_out[b][i, :, :])
                ats[(b, i)] = at

        # ---------------- emission ----------------
        with nc.named_scope("A"):
            xof0 = load_half(0)
            for kind in ("k", "v"):
                for b in range(NH):
                    proj(kind, b, xof0, 0)
            for b in range(NH):
                proj("q", b, xof0, 0)
            xof1 = load_half(1)
            proj("k", 0, xof1, HALF)
            proj("v", 0, xof1, HALF)
            transp(0)
            proj("k", 1, xof1, HALF)
            proj("v", 1, xof1, HALF)
            transp(1)
            proj("q", 0, xof1, HALF)

        attn_blk(0, 0)
        with nc.named_scope("A"):
            proj("q", 1, xof1, HALF)
        xpool.release()
        wpool.release()
        psA.release()
        # prefetch head-1 wo row-blocks during attention (head-0's load
        # inline in phase D, overlapping the second AllToAll)
        wo_tiles = {}
        for h in range(1, 2 * W, 2):
            wo_sb = dpool.tile([P, D], f16, name=f"wo{h}", tag=f"wo{h}")
            nc.scalar.dma_start(wo_sb[:], wo_t[h * P:(h + 1) * P, :])
            wo_tiles[h] = wo_sb
        attn_blk(0, 1)
        a2a(0)
        attn_blk(1, 0)
        attn_blk(1, 1)
        a2a(1)
        psPV.release()
        psS.release()

        # ---------------- Phase D: output projection ----------------
        psD = tc.alloc_tile_pool(name="psD", bufs=1, space="PSUM")
        ps_y = [[psD.tile([P, 512], f32, name=f"ps_y{qb}_{dc}",
                          tag=f"ps_y{qb}_{dc}")
                 for dc in range(4)] for qb in range(2)]
        first = True
        for b in range(NH):
            for i in range(W):
                at = ats[(b, i)]
                h = 2 * i + b
                if h in wo_tiles:
                    wo_sb = wo_tiles[h]
                else:
                    wo_sb = dpool.tile([P, D], f16, name=f"wo{h}",
                                       tag="wo_e", bufs=6)
                    nc.scalar.dma_start(wo_sb[:], wo_t[h * P:(h + 1) * P, :])
                last = (b == NH - 1) and (i == W - 1)
                for qb in range(2):
                    for dc in range(4):
                        nc.tensor.matmul(ps_y[qb][dc][:],
                                         at[:, qb * P:(qb + 1) * P],
                                         wo_sb[:, dc * 512:(dc + 1) * 512],
                                         start=first, stop=last)
                first = False
        for qb in range(2):
            for dc in range(4):
                y_sb = dpool.tile([P, 512], f32, name="y_sb", tag="y_sb",
                                  bufs=3)
                nc.vector.tensor_copy(out=y_sb[:], in_=ps_y[qb][dc][:])
                nc.gpsimd.dma_start(
                    y[qb * P:(qb + 1) * P, dc * 512:(dc + 1) * 512], y_sb[:])
        psD.release()

        for p in [dpool, vpool, maskp, dram, smallp, probsp, consts, persist]:
            p.release()

    nc.compile()
    return nc


def _warr(w, scale=None):
    """[D, 256] -> [NH, P, DT, HD] host layout (contiguous per head)."""
    wt = w.T if scale is None else (w * scale).T          # [D, 256]
    a = wt.reshape(DT, P, NH, HD).transpose(2, 1, 0, 3)   # [NH, P, DT, HD]
    return np.ascontiguousarray(a).astype(np.float16)


def make_in_maps(x, mask, wq, wk, wv, wo):
    """x [1,S,D]; mask [1,16,S,S]; w* [D,D] (all f32) -> per-core dicts."""
    scale = np.float32(1.0 / math.sqrt(HD))
    xtv = x[0].T.astype(np.float16)                       # [D, S]
    # [half, g, p, w, col] grouped-DMA layout
    xg = np.ascontiguousarray(
        xtv.reshape(XG, XW, P, 2, HALF).transpose(3, 0, 2, 1, 4))
    wo_tv = np.ascontiguousarray(wo.T).astype(np.float16)
    in_maps = []
    for c in range(W):
        rows = slice(NH * HD * c, NH * HD * (c + 1))
        m = mask[0, NH * c:NH * (c + 1)]
        em = np.exp(m.transpose(0, 2, 1))
        in_maps.append({
            "xt": xg,
            "wq_t": _warr(wq[rows], scale),
            "wk_t": _warr(wk[rows]),
            "wv_t": _warr(wv[rows]),
            "emask_t": np.ascontiguousarray(em).astype(np.float16),
            "wo_t": wo_tv,
        })
    return in_maps


def assemble(results):
    return np.concatenate([results[c]["y"] for c in range(W)], axis=0)[None]


# ----------------------------------------------------------------------
# Harness entry point: kernel(**inputs) takes the FULL unsharded inputs
# as produced by setup_inputs() and returns the FULL [1, S, D] output.
# Inside: inputs are sharded head-wise (TP) across the 8 NeuronCores,
# the Bass kernel runs SPMD (with two AllToAll collectives), and the
# seq-sharded outputs are concatenated on the host.
# ----------------------------------------------------------------------
_NC_CACHE = []


def kernel(x, mask, start_pos, wq, wk, wv, wo):
    from concourse import bass_utils
    x = np.asarray(x, dtype=np.float32)
    mask = np.asarray(mask, dtype=np.float32)
    wq = np.asarray(wq, dtype=np.float32)
    wk = np.asarray(wk, dtype=np.float32)
    wv = np.asarray(wv, dtype=np.float32)
    wo = np.asarray(wo, dtype=np.float32)
    # start_pos == 0 prefill (as in the reference)
    if not _NC_CACHE:
        _NC_CACHE.append(build())
    nc = _NC_CACHE[0]
    in_maps = make_in_maps(x, mask, wq, wk, wv, wo)
    res = bass_utils.run_bass_kernel_spmd(nc, in_maps, core_ids=list(range(W)))
    return assemble(res.results).astype(np.float32)
